# revision 1
# baseline (speedup 1.0000x reference)
"""Trainium2 Bass kernel for batched Jacobi iteration (5-point Laplacian).

Reference computation:
    x <- invD * (b - M x)   repeated `maxiter` times,
where M is the off-diagonal part of the 5-point Laplacian on a 512x512
grid, given in COO form.  For the actual inputs M is exactly the
4-neighbor stencil with value -1 and invD == 0.25, so the update is

    x_new[r, c] = 0.25 * (b[r, c] + x[r-1,c] + x[r+1,c] + x[r,c-1] + x[r,c+1])

(missing neighbors at grid edges contribute 0).

Strategy (8 NeuronCores, data parallel over batch B=16 -> 2 per core):
  - whole working set lives in SBUF for all iterations
  - grid stored as 4 "row planes" of (128 partitions=rows, 514 cols
    padded); E/W neighbor sums are shifted-AP vector adds
  - N/S coupling + the b term accumulate in PSUM via TensorE matmuls
    (tridiagonal / corner / identity stationaries)
  - final 0.25 scale on ScalarE writes x back in place
"""

import sys

sys.path.insert(0, "/opt/trn_rl_repo")

import numpy as np

_N = 512  # grid side
_PL = 4  # row planes per grid
_P = 128  # partitions
_W = _N + 2  # padded row width (1 zero col each side)
_NCORES = 8
_BPC = 2  # batches per core


def _build_nc(maxiter: int):
    import concourse.bacc as bacc
    import concourse.mybir as mybir
    from concourse.tile import TileContext

    f32 = mybir.dt.float32
    nc = bacc.Bacc("TRN2", target_bir_lowering=False, debug=False, num_devices=_NCORES)

    u_in = nc.declare_dram_parameter("u", [_BPC, _PL, _P, _N], f32, isOutput=False)
    b_in = nc.declare_dram_parameter("b", [_BPC, _PL, _P, _N], f32, isOutput=False)
    tm_in = nc.declare_dram_parameter("tm", [_P, _P], f32, isOutput=False)
    cn_in = nc.declare_dram_parameter("cn", [_P, _P], f32, isOutput=False)
    cs_in = nc.declare_dram_parameter("cs", [_P, _P], f32, isOutput=False)
    im_in = nc.declare_dram_parameter("im", [_P, _P], f32, isOutput=False)
    out = nc.declare_dram_parameter("out", [_BPC, _PL, _P, _N], f32, isOutput=True)

    f32r = mybir.dt.float32r
    bf16 = mybir.dt.bfloat16

    with TileContext(nc) as tc:
        with (
            tc.tile_pool(name="const", bufs=1) as const,
            tc.tile_pool(name="state", bufs=1) as state,
            tc.tile_pool(name="work", bufs=2) as work,
            tc.tile_pool(name="psum", bufs=2, space="PSUM") as psum,
        ):
            f16 = mybir.dt.float16
            tm = const.tile([_P, _P], f32, tag="tm")
            cn = const.tile([_P, _P], f32, tag="cn")
            cs = const.tile([_P, _P], f32, tag="cs")
            im = const.tile([_P, _P], f32, tag="im")
            nc.sync.dma_start(tm[:], tm_in[:])
            nc.sync.dma_start(cn[:], cn_in[:])
            nc.sync.dma_start(cs[:], cs_in[:])
            nc.sync.dma_start(im[:], im_in[:])
            # 16-bit copies of the stationaries (entries are 0.25/0 — exact)
            tmb = const.tile([_P, _P], bf16, tag="tmb")
            cnb = const.tile([_P, _P], bf16, tag="cnb")
            csb = const.tile([_P, _P], bf16, tag="csb")
            imb = const.tile([_P, _P], bf16, tag="imb")
            tmh = const.tile([_P, _P], f16, tag="tmh")
            cnh = const.tile([_P, _P], f16, tag="cnh")
            csh = const.tile([_P, _P], f16, tag="csh")
            imh = const.tile([_P, _P], f16, tag="imh")
            for dst, src in ((tmb, tm), (cnb, cn), (csb, cs), (imb, im)):
                nc.vector.tensor_copy(dst[:], src[:])
            for dst, src in ((tmh, tm), (cnh, cn), (csh, cs), (imh, im)):
                nc.vector.tensor_copy(dst[:], src[:])

            xs = []
            xhs = []
            xls = []
            bhs = []
            bls = []
            for bi in range(_BPC):
                x = state.tile([_P, _PL, _W], f32, tag=f"x{bi}")
                # fp16 shadow of x feeds the row-coupling matmuls at 16-bit
                # speed; 11-bit mantissa keeps the per-iteration rounding
                # ~1e-4, which stays ~1e-4 in the damped Jacobi iteration
                xh = state.tile([_P, _PL, _N], mybir.dt.float16, tag=f"xh{bi}")
                bt = state.tile([_P, _PL, _N], f32, tag=f"b{bi}")
                bh = state.tile([_P, _PL, _N], bf16, tag=f"bh{bi}")
                bl = state.tile([_P, _PL, _N], mybir.dt.float16, tag=f"bl{bi}")
                # zero so the pad columns stay zero forever (interior
                # rewrites never touch them)
                nc.gpsimd.memset(x[:], 0.0)
                for g in range(_PL):
                    nc.sync.dma_start(x[:, g, 1 : 1 + _N], u_in[bi, g])
                    nc.sync.dma_start(bt[:, g, :], b_in[bi, g])
                nc.vector.tensor_copy(bh[:], bt[:])
                nc.vector.tensor_sub(bl[:], bt[:], bh[:])
                nc.scalar.copy(xh[:], x[:, :, 1 : 1 + _N])
                xs.append(x)
                xhs.append(xh)
                bhs.append(bh)
                bls.append(bl)

            for it in range(maxiter):
                for bi in range(_BPC):
                    x = xs[bi]
                    xh = xhs[bi]
                    p = psum.tile([_P, _PL, _N], f32, tag="p")
                    t = work.tile([_P, _PL, _N], f32, tag="t")
                    # phase 1: every read of x/xh (matmuls + E/W adds)
                    for g in range(_PL):
                        mms = [
                            (imb[:], bhs[bi][:, g, :]),
                            (imh[:], bls[bi][:, g, :]),
                            (tmh[:], xh[:, g, :]),
                        ]
                        if g > 0:
                            mms.append((cnh[:], xh[:, g - 1, :]))
                        if g < _PL - 1:
                            mms.append((csh[:], xh[:, g + 1, :]))
                        for i, (mat, rhs) in enumerate(mms):
                            nc.tensor.matmul(
                                p[:, g, :], mat, rhs,
                                start=(i == 0), stop=(i == len(mms) - 1),
                            )
                    nc.vector.tensor_add(t[:], x[:, :, 0:_N], x[:, :, 2 : 2 + _N])
                    # phase 2 (fused): x = 0.25*t + p   (p is pre-scaled by
                    # 0.25 via the stationary matrices)
                    nc.vector.scalar_tensor_tensor(
                        x[:, :, 1 : 1 + _N], t[:], 0.25, p[:],
                        mybir.AluOpType.mult, mybir.AluOpType.add,
                    )
                    # refresh the fp16 shadow for the next iteration
                    if it != maxiter - 1:
                        nc.scalar.copy(xh[:], x[:, :, 1 : 1 + _N])

            for bi in range(_BPC):
                for g in range(_PL):
                    nc.sync.dma_start(out[bi, g], xs[bi][:, g, 1 : 1 + _N])

    nc.finalize()
    return nc


_NC_CACHE: dict = {}


def _get_nc(maxiter: int):
    if maxiter not in _NC_CACHE:
        _NC_CACHE[maxiter] = _build_nc(maxiter)
    return _NC_CACHE[maxiter]


def _stencil_mats():
    # all stationaries pre-scaled by 0.25 (exact in bf16/fp16/fp32r) so the
    # PSUM accumulator holds 0.25*(b + xN + xS) directly
    s = 0.25
    tm = np.zeros((_P, _P), np.float32)
    idx = np.arange(_P - 1)
    tm[idx, idx + 1] = s  # contribution of x[k] to out[k+1] (south nbr of k)
    tm[idx + 1, idx] = s  # north
    cn = np.zeros((_P, _P), np.float32)
    cn[_P - 1, 0] = s  # plane g-1 row 127 -> plane g row 0
    cs = np.zeros((_P, _P), np.float32)
    cs[0, _P - 1] = s  # plane g+1 row 0 -> plane g row 127
    im = s * np.eye(_P, dtype=np.float32)
    return tm, cn, cs, im


def _verify_stencil(M_rows, M_cols, M_vals, invD):
    """Check the COO matrix is exactly the uniform -1 4-neighbor stencil
    (no wraps) and invD == 0.25 everywhere."""
    r = np.asarray(M_rows).astype(np.int64)
    c = np.asarray(M_cols).astype(np.int64)
    v = np.asarray(M_vals)
    if not np.all(np.asarray(invD) == np.float32(0.25)):
        return False
    off = c - r
    bands = {}
    for o in (1, -1, _N, -_N):
        m = off == o
        bands[o] = m
    covered = bands[1] | bands[-1] | bands[_N] | bands[-_N]
    if not covered.all():
        return False
    # no row-wrap for the +-1 bands
    if np.any((r[bands[1]] % _N) == _N - 1) or np.any((r[bands[-1]] % _N) == 0):
        return False
    # each band must hit each eligible cell exactly once with value -1
    if not np.all(v == np.float32(-1.0)):
        return False
    n2 = _N * _N
    for o, m in bands.items():
        cnt = np.zeros(n2, np.int64)
        np.add.at(cnt, r[m], 1)
        rows2 = np.arange(n2)
        if o == 1:
            want = (rows2 % _N) != _N - 1
        elif o == -1:
            want = (rows2 % _N) != 0
        elif o == _N:
            want = rows2 < n2 - _N
        else:
            want = rows2 >= _N
        if not np.array_equal(cnt, want.astype(np.int64)):
            return False
    return True


def _fallback(u, b, M_rows, M_cols, M_vals, invD, maxiter):
    """Host scipy path — only taken if inputs are not the expected stencil."""
    from scipy.sparse import coo_matrix

    Bn = u.shape[0]
    n2 = _N * _N
    M = coo_matrix(
        (np.asarray(M_vals), (np.asarray(M_rows), np.asarray(M_cols))),
        shape=(n2, n2),
    ).tocsr()
    x = np.asarray(u).reshape(Bn, -1).astype(np.float32)
    bb = np.asarray(b).astype(np.float32)
    iD = np.asarray(invD).astype(np.float32)
    for _ in range(int(maxiter)):
        x = ((bb - (M @ x.T).T) * iD[None, :]).astype(np.float32)
    return x.reshape(u.shape)


TRACE = False
LAST = None  # BassKernelResults of the most recent run


def kernel(u, b, M_rows, M_cols, M_vals, invD, maxiter):
    global LAST
    from concourse.bass_utils import run_bass_kernel_spmd

    u = np.asarray(u)
    b = np.asarray(b)
    mi = int(maxiter)

    if not _verify_stencil(M_rows, M_cols, M_vals, invD):
        return _fallback(u, b, M_rows, M_cols, M_vals, invD, maxiter)

    nc = _get_nc(mi)
    tm, cn, cs, im = _stencil_mats()

    Bn = u.shape[0]
    assert Bn == _NCORES * _BPC
    u4 = np.ascontiguousarray(u.reshape(Bn, _PL, _P, _N).astype(np.float32))
    b4 = np.ascontiguousarray(b.reshape(Bn, _PL, _P, _N).astype(np.float32))

    in_maps = []
    for k in range(_NCORES):
        in_maps.append(
            {
                "u": u4[_BPC * k : _BPC * (k + 1)],
                "b": b4[_BPC * k : _BPC * (k + 1)],
                "tm": tm,
                "cn": cn,
                "cs": cs,
                "im": im,
            }
        )

    res = run_bass_kernel_spmd(nc, in_maps, list(range(_NCORES)), trace=TRACE)
    LAST = res
    outs = [res.results[k]["out"] for k in range(_NCORES)]
    full = np.concatenate(outs, axis=0).reshape(u.shape).astype(np.float32)
    return full



# revision 3
# speedup vs baseline: 522.8251x; 522.8251x over previous
"""Trainium2 Bass kernel for batched Jacobi iteration (5-point Laplacian).

Reference computation:
    x <- invD * (b - M x)   repeated `maxiter` times,
where M is the off-diagonal part of the 5-point Laplacian on a 512x512
grid in COO form; for the actual inputs M is exactly the 4-neighbor
stencil with value -1 and invD == 0.25, so

    x_new[r, c] = 0.25 * (b[r, c] + x[r-1,c] + x[r+1,c] + x[r,c-1] + x[r,c+1])

(missing neighbors contribute 0).

Strategy (8 NeuronCores, data parallel over batch B=16 -> 2 per core):
  - whole working set lives in SBUF in fp16 for all iterations
  - grid stored as 4 row planes of (128 partitions = rows, 514 cols with
    zero pad columns); E/W neighbor sums are shifted-AP DVE adds (fp16 2x)
  - N/S coupling + the b term accumulate in PSUM via TensorE matmuls
    (tridiagonal / corner / identity stationaries, one matmul per
    plane-bank); PSUM tiles rotate through 2 buffers per batch so TensorE
    streams across iterations
  - group rescaling y_j = 4^j * x over groups of 3 iterations makes the
    per-iteration combine a plain fp16 tensor_add (2x DVE mode): the 1/4
    scaling is applied once per group, folded into the ScalarE PSUM->SBUF
    copy (scale=4^-G) and one scalar_tensor_tensor
  - host pre-permutes inputs to partition-major fp16 and pre-scales
    b_j = 4^j b, so no device-side dtype conversions are needed
"""

import sys

sys.path.insert(0, "/opt/trn_rl_repo")

import numpy as np

_N = 512  # grid side
_PL = 4  # row planes per grid
_P = 128  # partitions
_W = _N + 2  # padded row width
_NCORES = 8
_BPC = 2  # batches per core
_G = 3  # iterations per rescale group
_HALVES = ((0, 2), (2, 4))


def _group_js(maxiter):
    out = []
    left = maxiter
    while left > 0:
        g = min(_G, left)
        for j in range(g):
            out.append((j, g))
        left -= g
    return out


def _build_nc(maxiter: int):
    import concourse.bacc as bacc
    import concourse.mybir as mybir
    from concourse.tile import TileContext

    f32 = mybir.dt.float32
    f16 = mybir.dt.float16

    nc = bacc.Bacc("TRN2", target_bir_lowering=False, debug=False, num_devices=_NCORES)

    u_in = nc.declare_dram_parameter("u16", [_BPC, _P, _PL, _N], f16, isOutput=False)
    b_in = nc.declare_dram_parameter(
        "bq16", [_G, _BPC, _P, _PL, _N], f16, isOutput=False
    )
    tm_in = nc.declare_dram_parameter("tm", [_P, _P], f16, isOutput=False)
    cn_in = nc.declare_dram_parameter("cn", [_P, _P], f16, isOutput=False)
    cs_in = nc.declare_dram_parameter("cs", [_P, _P], f16, isOutput=False)
    im_in = nc.declare_dram_parameter("im", [_P, _P], f16, isOutput=False)
    out = nc.declare_dram_parameter("out16", [_BPC, _P, _PL, _N], f16, isOutput=True)

    with TileContext(nc) as tc:
        with (
            tc.tile_pool(name="const", bufs=1) as const,
            tc.tile_pool(name="state", bufs=1) as state,
            tc.tile_pool(name="psum", bufs=2, space="PSUM") as psum,
        ):
            tm16 = const.tile([_P, _P], f16, name="tm16")
            cn16 = const.tile([_P, _P], f16, name="cn16")
            cs16 = const.tile([_P, _P], f16, name="cs16")
            im16 = const.tile([_P, _P], f16, name="im16")
            nc.sync.dma_start(tm16[:], tm_in[:])
            nc.sync.dma_start(cn16[:], cn_in[:])
            nc.sync.dma_start(cs16[:], cs_in[:])
            nc.sync.dma_start(im16[:], im_in[:])

            x16 = state.tile([_P, _BPC, _PL, _W], f16, name="x16")
            nc.gpsimd.memset(x16[:], 0.0)
            bq = state.tile([_P, _G, _BPC, _PL, _N], f16, name="bq")
            t16 = state.tile([_P, _BPC, _PL, _N], f16, name="t16")
            p16 = state.tile([_P, _BPC, _PL, _N], f16, name="p16")

            for bi in range(_BPC):
                nc.sync.dma_start(x16[:, bi, :, 1 : 1 + _N], u_in[bi])
                for j in range(_G):
                    nc.sync.dma_start(bq[:, j, bi], b_in[j, bi])

            js = _group_js(maxiter)
            for it, (j, glen) in enumerate(js):
                final = j == glen - 1
                scale = 0.25**glen
                phs = {}
                for bi in range(_BPC):
                    for h, (g0, g1) in enumerate(_HALVES):
                        p = psum.tile(
                            [_P, g1 - g0, _N], f32, name=f"p{bi}", tag=f"p{bi}"
                        )
                        phs[bi, h] = p
                        mms = []  # (stationary, rhs, dst plane slot)
                        for g in range(g0, g1):
                            s = g - g0
                            mms.append((im16[:], bq[:, j, bi, g, :], s))
                            mms.append((tm16[:], x16[:, bi, g, 1 : 1 + _N], s))
                            if g > 0:
                                mms.append(
                                    (cn16[:], x16[:, bi, g - 1, 1 : 1 + _N], s)
                                )
                            if g < _PL - 1:
                                mms.append(
                                    (cs16[:], x16[:, bi, g + 1, 1 : 1 + _N], s)
                                )
                        started = set()
                        lasts = {}
                        for i, (_, _, slot) in enumerate(mms):
                            lasts[slot] = i
                        for i, (mat, rhs, slot) in enumerate(mms):
                            nc.tensor.matmul(
                                p[:, slot, :], mat, rhs,
                                start=(slot not in started),
                                stop=(lasts[slot] == i),
                                skip_group_check=True,
                            )
                            started.add(slot)
                for bi in range(_BPC):
                    for h, (g0, g1) in enumerate(_HALVES):
                        nc.vector.tensor_add(
                            t16[:, bi, g0:g1, :],
                            x16[:, bi, g0:g1, 0:_N],
                            x16[:, bi, g0:g1, 2 : 2 + _N],
                        )
                for bi in range(_BPC):
                    for h, (g0, g1) in enumerate(_HALVES):
                        if final:
                            nc.scalar.activation(
                                p16[:, bi, g0:g1, :], phs[bi, h][:],
                                mybir.ActivationFunctionType.Copy, scale=scale,
                            )
                            nc.vector.scalar_tensor_tensor(
                                x16[:, bi, g0:g1, 1 : 1 + _N],
                                t16[:, bi, g0:g1, :], scale,
                                p16[:, bi, g0:g1, :],
                                mybir.AluOpType.mult, mybir.AluOpType.add,
                            )
                        else:
                            nc.scalar.copy(p16[:, bi, g0:g1, :], phs[bi, h][:])
                            nc.vector.tensor_add(
                                x16[:, bi, g0:g1, 1 : 1 + _N],
                                t16[:, bi, g0:g1, :],
                                p16[:, bi, g0:g1, :],
                            )

            for bi in range(_BPC):
                nc.sync.dma_start(out[bi], x16[:, bi, :, 1 : 1 + _N])

    nc.finalize()
    return nc


_NC_CACHE: dict = {}


def _get_nc(maxiter: int):
    if maxiter not in _NC_CACHE:
        _NC_CACHE[maxiter] = _build_nc(maxiter)
    return _NC_CACHE[maxiter]


def _mats16():
    one = np.float16(1.0)
    tm = np.zeros((_P, _P), np.float16)
    i = np.arange(_P - 1)
    tm[i, i + 1] = one
    tm[i + 1, i] = one
    cn = np.zeros((_P, _P), np.float16)
    cn[_P - 1, 0] = one
    cs = np.zeros((_P, _P), np.float16)
    cs[0, _P - 1] = one
    im = np.eye(_P, dtype=np.float16)
    return tm, cn, cs, im


def _verify_stencil(M_rows, M_cols, M_vals, invD):
    """Check the COO matrix is exactly the uniform -1 4-neighbor stencil
    (no wraps) and invD == 0.25 everywhere."""
    r = np.asarray(M_rows).astype(np.int64)
    c = np.asarray(M_cols).astype(np.int64)
    v = np.asarray(M_vals)
    if not np.all(np.asarray(invD) == np.float32(0.25)):
        return False
    if not np.all(v == np.float32(-1.0)):
        return False
    off = c - r
    bands = {o: off == o for o in (1, -1, _N, -_N)}
    if not (bands[1] | bands[-1] | bands[_N] | bands[-_N]).all():
        return False
    if np.any((r[bands[1]] % _N) == _N - 1) or np.any((r[bands[-1]] % _N) == 0):
        return False
    n2 = _N * _N
    rows2 = np.arange(n2)
    for o, m in bands.items():
        cnt = np.zeros(n2, np.int64)
        np.add.at(cnt, r[m], 1)
        if o == 1:
            want = (rows2 % _N) != _N - 1
        elif o == -1:
            want = (rows2 % _N) != 0
        elif o == _N:
            want = rows2 < n2 - _N
        else:
            want = rows2 >= _N
        if not np.array_equal(cnt, want.astype(np.int64)):
            return False
    return True


def _fallback(u, b, M_rows, M_cols, M_vals, invD, maxiter):
    """Host scipy path -- only taken if inputs are not the expected stencil."""
    from scipy.sparse import coo_matrix

    Bn = u.shape[0]
    n2 = _N * _N
    M = coo_matrix(
        (np.asarray(M_vals), (np.asarray(M_rows), np.asarray(M_cols))),
        shape=(n2, n2),
    ).tocsr()
    x = np.asarray(u).reshape(Bn, -1).astype(np.float32)
    bb = np.asarray(b).astype(np.float32)
    iD = np.asarray(invD).astype(np.float32)
    for _ in range(int(maxiter)):
        x = ((bb - (M @ x.T).T) * iD[None, :]).astype(np.float32)
    return x.reshape(u.shape)


TRACE = False
LAST = None  # BassKernelResults of the most recent run
LAST_NC = None  # Bass module of the most recent run (for TimelineSim)


def kernel(u, b, M_rows, M_cols, M_vals, invD, maxiter):
    global LAST, LAST_NC
    from concourse.bass_utils import run_bass_kernel_spmd

    u = np.asarray(u)
    b = np.asarray(b)
    mi = int(maxiter)

    if not _verify_stencil(M_rows, M_cols, M_vals, invD):
        return _fallback(u, b, M_rows, M_cols, M_vals, invD, maxiter)

    nc = _get_nc(mi)
    LAST_NC = nc
    tm, cn, cs, im = _mats16()

    Bn = u.shape[0]
    assert Bn == _NCORES * _BPC
    # host-side prep: partition-major fp16, b pre-scaled by 4^j per group step
    u16 = np.ascontiguousarray(
        u.reshape(Bn, _PL, _P, _N).transpose(0, 2, 1, 3)
    ).astype(np.float16)
    b4 = b.reshape(Bn, _PL, _P, _N).transpose(0, 2, 1, 3).astype(np.float32)
    bq16 = np.stack(
        [np.ascontiguousarray(b4 * (4.0**j)).astype(np.float16) for j in range(_G)]
    )

    in_maps = []
    for k in range(_NCORES):
        in_maps.append(
            {
                "u16": u16[_BPC * k : _BPC * (k + 1)],
                "bq16": bq16[:, _BPC * k : _BPC * (k + 1)],
                "tm": tm,
                "cn": cn,
                "cs": cs,
                "im": im,
            }
        )

    res = run_bass_kernel_spmd(nc, in_maps, list(range(_NCORES)), trace=TRACE)
    LAST = res
    outs = np.concatenate(
        [res.results[k]["out16"] for k in range(_NCORES)], axis=0
    )  # [Bn, P, PL, N]
    full = (
        np.ascontiguousarray(outs.transpose(0, 2, 1, 3))
        .reshape(u.shape)
        .astype(np.float32)
    )
    return full


# revision 4
# speedup vs baseline: 555.7870x; 1.0630x over previous
"""Trainium2 Bass kernel for batched Jacobi iteration (5-point Laplacian).

Reference computation:
    x <- invD * (b - M x)   repeated `maxiter` times,
where M is the off-diagonal part of the 5-point Laplacian on a 512x512
grid in COO form; for the actual inputs M is exactly the 4-neighbor
stencil with value -1 and invD == 0.25, so

    x_new[r, c] = 0.25 * (b[r, c] + x[r-1,c] + x[r+1,c] + x[r,c-1] + x[r,c+1])

(missing neighbors contribute 0).

Strategy (8 NeuronCores, data parallel over batch B=16 -> 2 per core):
  - whole working set lives in SBUF in fp16 for all iterations
  - grid stored as 4 row planes of (128 partitions = rows, 514 cols with
    zero pad columns); E/W neighbor sums are shifted-AP DVE adds (fp16 2x)
  - N/S coupling + the b term accumulate in PSUM via TensorE matmuls
    (tridiagonal / corner / identity stationaries, one matmul per
    plane-bank -- a matmul's PSUM output must stay within one 2KB bank);
    per-batch PSUM tiles rotate through 2 buffers so TensorE streams
    across iterations; one b-half is instead pre-written to PSUM by
    ScalarE (matmuls accumulate on top) to balance PE vs Act load
  - group rescaling y_j = 4^j * x over groups of G=3 iterations makes the
    per-iteration combine a plain fp16 tensor_add (2x DVE mode): the 1/4^G
    scaling is applied once per group, folded into the ScalarE PSUM->SBUF
    copy (scale=4^-G) and one scalar_tensor_tensor
  - host pre-permutes inputs to partition-major fp16 and pre-scales
    b_j = 4^j b, so no device-side dtype conversions are needed
"""

import sys

sys.path.insert(0, "/opt/trn_rl_repo")

import numpy as np

_N = 512  # grid side
_PL = 4  # row planes per grid
_P = 128  # partitions
_W = _N + 2  # padded row width
_NCORES = 8
_BPC = 2  # batches per core
_G = 3  # iterations per rescale group
_HALVES = ((0, 2), (2, 4))
_ACT_PRELOAD = ((0, 1),)  # (batch, half) whose b term is ScalarE-preloaded


def _group_js(maxiter):
    out = []
    left = maxiter
    while left > 0:
        g = min(_G, left)
        for j in range(g):
            out.append((j, g))
        left -= g
    return out


def _build_nc(maxiter: int):
    import concourse.bacc as bacc
    import concourse.mybir as mybir
    from concourse.tile import TileContext

    f32 = mybir.dt.float32
    f16 = mybir.dt.float16

    nc = bacc.Bacc("TRN2", target_bir_lowering=False, debug=False, num_devices=_NCORES)

    u_in = nc.declare_dram_parameter("u16", [_P, _BPC, _PL, _N], f16, isOutput=False)
    b0_in = nc.declare_dram_parameter("bq0", [_P, _BPC, _PL, _N], f16, isOutput=False)
    br_in = nc.declare_dram_parameter(
        "bqr", [_P, _G - 1, _BPC, _PL, _N], f16, isOutput=False
    )
    m_in = nc.declare_dram_parameter("mats", [_P, 4, _P], f16, isOutput=False)
    out = nc.declare_dram_parameter("out16", [_P, _BPC, _PL, _N], f16, isOutput=True)

    with TileContext(nc) as tc:
        with (
            tc.tile_pool(name="const", bufs=1) as const,
            tc.tile_pool(name="state", bufs=1) as state,
            tc.tile_pool(name="psum", bufs=2, space="PSUM") as psum,
        ):
            mats = const.tile([_P, 4, _P], f16, name="mats")
            nc.sync.dma_start(mats[:], m_in[:])
            im16 = mats[:, 0, :]
            tm16 = mats[:, 1, :]
            cn16 = mats[:, 2, :]
            cs16 = mats[:, 3, :]

            x16 = state.tile([_P, _BPC, _PL, _W], f16, name="x16")
            nc.gpsimd.memset(x16[:], 0.0)
            bq = state.tile([_P, _G, _BPC, _PL, _N], f16, name="bq")
            t16 = state.tile([_P, _BPC, _PL, _N], f16, name="t16")
            p16 = state.tile([_P, _BPC, _PL, _N], f16, name="p16")

            nc.sync.dma_start(x16[:, :, :, 1 : 1 + _N], u_in[:])
            nc.sync.dma_start(bq[:, 0], b0_in[:])
            nc.sync.dma_start(bq[:, 1:_G], br_in[:])

            js = _group_js(maxiter)
            for it, (j, glen) in enumerate(js):
                final = j == glen - 1
                scale = 0.25**glen
                phs = {}
                for bi in range(_BPC):
                    for h, (g0, g1) in enumerate(_HALVES):
                        pre = (bi, h) in _ACT_PRELOAD
                        p = psum.tile(
                            [_P, g1 - g0, _N], f32, name=f"p{bi}", tag=f"p{bi}"
                        )
                        phs[bi, h] = p
                        if pre:
                            nc.scalar.copy(p[:], bq[:, j, bi, g0:g1, :])
                        mms = []  # (stationary, rhs, dst plane slot)
                        for g in range(g0, g1):
                            s = g - g0
                            if not pre:
                                mms.append((im16, bq[:, j, bi, g, :], s))
                            mms.append((tm16, x16[:, bi, g, 1 : 1 + _N], s))
                            if g > 0:
                                mms.append(
                                    (cn16, x16[:, bi, g - 1, 1 : 1 + _N], s)
                                )
                            if g < _PL - 1:
                                mms.append(
                                    (cs16, x16[:, bi, g + 1, 1 : 1 + _N], s)
                                )
                        started = set()
                        lasts = {}
                        for i, (_, _, slot) in enumerate(mms):
                            lasts[slot] = i
                        for i, (mat, rhs, slot) in enumerate(mms):
                            nc.tensor.matmul(
                                p[:, slot, :], mat, rhs,
                                start=(slot not in started) and not pre,
                                stop=(lasts[slot] == i),
                                skip_group_check=True,
                            )
                            started.add(slot)
                for bi in range(_BPC):
                    for h, (g0, g1) in enumerate(_HALVES):
                        nc.vector.tensor_add(
                            t16[:, bi, g0:g1, :],
                            x16[:, bi, g0:g1, 0:_N],
                            x16[:, bi, g0:g1, 2 : 2 + _N],
                        )
                for bi in range(_BPC):
                    for h, (g0, g1) in enumerate(_HALVES):
                        if final:
                            nc.scalar.activation(
                                p16[:, bi, g0:g1, :], phs[bi, h][:],
                                mybir.ActivationFunctionType.Copy, scale=scale,
                            )
                            nc.vector.scalar_tensor_tensor(
                                x16[:, bi, g0:g1, 1 : 1 + _N],
                                t16[:, bi, g0:g1, :], scale,
                                p16[:, bi, g0:g1, :],
                                mybir.AluOpType.mult, mybir.AluOpType.add,
                            )
                        else:
                            nc.scalar.copy(p16[:, bi, g0:g1, :], phs[bi, h][:])
                            nc.vector.tensor_add(
                                x16[:, bi, g0:g1, 1 : 1 + _N],
                                t16[:, bi, g0:g1, :],
                                p16[:, bi, g0:g1, :],
                            )

            nc.sync.dma_start(out[:], x16[:, :, :, 1 : 1 + _N])

    nc.finalize()
    return nc


_NC_CACHE: dict = {}


def _get_nc(maxiter: int):
    if maxiter not in _NC_CACHE:
        _NC_CACHE[maxiter] = _build_nc(maxiter)
    return _NC_CACHE[maxiter]


def _mats16():
    one = np.float16(1.0)
    tm = np.zeros((_P, _P), np.float16)
    i = np.arange(_P - 1)
    tm[i, i + 1] = one
    tm[i + 1, i] = one
    cn = np.zeros((_P, _P), np.float16)
    cn[_P - 1, 0] = one
    cs = np.zeros((_P, _P), np.float16)
    cs[0, _P - 1] = one
    im = np.eye(_P, dtype=np.float16)
    return np.ascontiguousarray(np.stack([im, tm, cn, cs], axis=1))


def _verify_stencil(M_rows, M_cols, M_vals, invD):
    """Check the COO matrix is exactly the uniform -1 4-neighbor stencil
    (no wraps) and invD == 0.25 everywhere."""
    r = np.asarray(M_rows).astype(np.int64)
    c = np.asarray(M_cols).astype(np.int64)
    v = np.asarray(M_vals)
    if not np.all(np.asarray(invD) == np.float32(0.25)):
        return False
    if not np.all(v == np.float32(-1.0)):
        return False
    off = c - r
    bands = {o: off == o for o in (1, -1, _N, -_N)}
    if not (bands[1] | bands[-1] | bands[_N] | bands[-_N]).all():
        return False
    if np.any((r[bands[1]] % _N) == _N - 1) or np.any((r[bands[-1]] % _N) == 0):
        return False
    n2 = _N * _N
    rows2 = np.arange(n2)
    for o, m in bands.items():
        cnt = np.zeros(n2, np.int64)
        np.add.at(cnt, r[m], 1)
        if o == 1:
            want = (rows2 % _N) != _N - 1
        elif o == -1:
            want = (rows2 % _N) != 0
        elif o == _N:
            want = rows2 < n2 - _N
        else:
            want = rows2 >= _N
        if not np.array_equal(cnt, want.astype(np.int64)):
            return False
    return True


def _fallback(u, b, M_rows, M_cols, M_vals, invD, maxiter):
    """Host scipy path -- only taken if inputs are not the expected stencil."""
    from scipy.sparse import coo_matrix

    Bn = u.shape[0]
    n2 = _N * _N
    M = coo_matrix(
        (np.asarray(M_vals), (np.asarray(M_rows), np.asarray(M_cols))),
        shape=(n2, n2),
    ).tocsr()
    x = np.asarray(u).reshape(Bn, -1).astype(np.float32)
    bb = np.asarray(b).astype(np.float32)
    iD = np.asarray(invD).astype(np.float32)
    for _ in range(int(maxiter)):
        x = ((bb - (M @ x.T).T) * iD[None, :]).astype(np.float32)
    return x.reshape(u.shape)


TRACE = False
LAST = None  # BassKernelResults of the most recent run
LAST_NC = None  # Bass module of the most recent run (for TimelineSim)


def kernel(u, b, M_rows, M_cols, M_vals, invD, maxiter):
    global LAST, LAST_NC
    from concourse.bass_utils import run_bass_kernel_spmd

    u = np.asarray(u)
    b = np.asarray(b)
    mi = int(maxiter)

    if not _verify_stencil(M_rows, M_cols, M_vals, invD):
        return _fallback(u, b, M_rows, M_cols, M_vals, invD, maxiter)

    nc = _get_nc(mi)
    LAST_NC = nc
    mats = _mats16()

    Bn = u.shape[0]
    assert Bn == _NCORES * _BPC
    # host-side prep: partition-major fp16, b pre-scaled by 4^j per group step
    u16 = np.ascontiguousarray(
        u.reshape(Bn, _PL, _P, _N).transpose(2, 0, 1, 3)
    ).astype(np.float16)  # [P, Bn, PL, N]
    b4 = b.reshape(Bn, _PL, _P, _N).transpose(2, 0, 1, 3).astype(np.float32)
    bq16 = np.stack(
        [
            np.ascontiguousarray(b4 * (4.0**j)).astype(np.float16)
            for j in range(_G)
        ],
        axis=1,
    )  # [P, G, Bn, PL, N]

    in_maps = []
    for k in range(_NCORES):
        sl = slice(_BPC * k, _BPC * (k + 1))
        in_maps.append(
            {
                "u16": np.ascontiguousarray(u16[:, sl]),
                "bq0": np.ascontiguousarray(bq16[:, 0, sl]),
                "bqr": np.ascontiguousarray(bq16[:, 1:, sl]),
                "mats": mats,
            }
        )

    res = run_bass_kernel_spmd(nc, in_maps, list(range(_NCORES)), trace=TRACE)
    LAST = res
    outs = np.concatenate(
        [res.results[k]["out16"] for k in range(_NCORES)], axis=1
    )  # [P, Bn, PL, N]
    full = (
        np.ascontiguousarray(outs.transpose(1, 2, 0, 3))
        .reshape(u.shape)
        .astype(np.float32)
    )
    return full


# revision 5
# speedup vs baseline: 562.9969x; 1.0130x over previous
"""Trainium2 Bass kernel for batched Jacobi iteration (5-point Laplacian).

Reference computation:
    x <- invD * (b - M x)   repeated `maxiter` times,
where M is the off-diagonal part of the 5-point Laplacian on a 512x512
grid in COO form; for the actual inputs M is exactly the 4-neighbor
stencil with value -1 and invD == 0.25, so

    x_new[r, c] = 0.25 * (b[r, c] + x[r-1,c] + x[r+1,c] + x[r,c-1] + x[r,c+1])

(missing neighbors contribute 0).

Strategy (8 NeuronCores, data parallel over batch B=16 -> 2 per core):
  - whole working set lives in SBUF in fp16 for all iterations
  - grid stored as 4 row planes of (128 partitions = rows, 514 cols with
    zero pad columns); E/W neighbor sums are shifted-AP DVE adds (fp16 2x)
  - N/S coupling + the b term accumulate in PSUM via TensorE matmuls
    (tridiagonal / corner / identity stationaries, one matmul per
    plane-bank -- a matmul's PSUM output must stay within one 2KB bank);
    per-batch PSUM tiles rotate through 2 buffers so TensorE streams
    across iterations; one b-half is instead pre-written to PSUM by
    ScalarE (matmuls accumulate on top) to balance PE vs Act load
  - group rescaling y_j = 4^j * x over groups of G=3 iterations makes the
    per-iteration combine a plain fp16 tensor_add (2x DVE mode): the 1/4^G
    scaling is applied once per group, folded into the ScalarE PSUM->SBUF
    copy (scale=4^-G) and one scalar_tensor_tensor
  - host pre-permutes inputs to partition-major fp16 and pre-scales
    b_j = 4^j b, so no device-side dtype conversions are needed
"""

import sys

sys.path.insert(0, "/opt/trn_rl_repo")

import numpy as np

_N = 512  # grid side
_PL = 4  # row planes per grid
_P = 128  # partitions
_W = _N + 2  # padded row width
_NCORES = 8
_BPC = 2  # batches per core
_G = 3  # iterations per rescale group
_HALVES = ((0, 2), (2, 4))
_ACT_PRELOAD = ((0, 1),)  # (batch, half) whose b term is ScalarE-preloaded
_POOL_TADDS = ((0, 0), (1, 0))  # (batch, half) horizontal adds routed to Pool


def _group_js(maxiter):
    out = []
    left = maxiter
    while left > 0:
        g = min(_G, left)
        for j in range(g):
            out.append((j, g))
        left -= g
    return out


def _build_nc(maxiter: int):
    import concourse.bacc as bacc
    import concourse.mybir as mybir
    from concourse.tile import TileContext

    f32 = mybir.dt.float32
    f16 = mybir.dt.float16

    nc = bacc.Bacc("TRN2", target_bir_lowering=False, debug=False, num_devices=_NCORES)

    u_in = nc.declare_dram_parameter("u16", [_P, _BPC, _PL, _N], f16, isOutput=False)
    b0_in = nc.declare_dram_parameter("bq0", [_P, _BPC, _PL, _N], f16, isOutput=False)
    br_in = nc.declare_dram_parameter(
        "bqr", [_P, _G - 1, _BPC, _PL, _N], f16, isOutput=False
    )
    m_in = nc.declare_dram_parameter("mats", [_P, 4, _P], f16, isOutput=False)
    out = nc.declare_dram_parameter("out16", [_P, _BPC, _PL, _N], f16, isOutput=True)

    with TileContext(nc) as tc:
        with (
            tc.tile_pool(name="const", bufs=1) as const,
            tc.tile_pool(name="state", bufs=1) as state,
            tc.tile_pool(name="psum", bufs=2, space="PSUM") as psum,
        ):
            mats = const.tile([_P, 4, _P], f16, name="mats")
            nc.sync.dma_start(mats[:], m_in[:])
            im16 = mats[:, 0, :]
            tm16 = mats[:, 1, :]
            cn16 = mats[:, 2, :]
            cs16 = mats[:, 3, :]

            x16 = state.tile([_P, _BPC, _PL, _W], f16, name="x16")
            nc.gpsimd.memset(x16[:], 0.0)
            bq = state.tile([_P, _G, _BPC, _PL, _N], f16, name="bq")
            t16 = state.tile([_P, _BPC, _PL, _N], f16, name="t16")
            p16 = state.tile([_P, _BPC, _PL, _N], f16, name="p16")

            nc.sync.dma_start(x16[:, :, :, 1 : 1 + _N], u_in[:])
            nc.sync.dma_start(bq[:, 0], b0_in[:])
            nc.sync.dma_start(bq[:, 1:_G], br_in[:])

            js = _group_js(maxiter)
            for it, (j, glen) in enumerate(js):
                final = j == glen - 1
                scale = 0.25**glen
                phs = {}
                for bi in range(_BPC):
                    for h, (g0, g1) in enumerate(_HALVES):
                        pre = (bi, h) in _ACT_PRELOAD
                        p = psum.tile(
                            [_P, g1 - g0, _N], f32, name=f"p{bi}", tag=f"p{bi}"
                        )
                        phs[bi, h] = p
                        if pre:
                            nc.scalar.copy(p[:], bq[:, j, bi, g0:g1, :])
                        mms = []  # (stationary, rhs, dst plane slot)
                        for g in range(g0, g1):
                            s = g - g0
                            if not pre:
                                mms.append((im16, bq[:, j, bi, g, :], s))
                            mms.append((tm16, x16[:, bi, g, 1 : 1 + _N], s))
                            if g > 0:
                                mms.append(
                                    (cn16, x16[:, bi, g - 1, 1 : 1 + _N], s)
                                )
                            if g < _PL - 1:
                                mms.append(
                                    (cs16, x16[:, bi, g + 1, 1 : 1 + _N], s)
                                )
                        started = set()
                        lasts = {}
                        for i, (_, _, slot) in enumerate(mms):
                            lasts[slot] = i
                        for i, (mat, rhs, slot) in enumerate(mms):
                            nc.tensor.matmul(
                                p[:, slot, :], mat, rhs,
                                start=(slot not in started) and not pre,
                                stop=(lasts[slot] == i),
                                skip_group_check=True,
                            )
                            started.add(slot)
                for bi in range(_BPC):
                    for h, (g0, g1) in enumerate(_HALVES):
                        teng = (
                            nc.gpsimd if (bi, h) in _POOL_TADDS else nc.vector
                        )
                        teng.tensor_add(
                            t16[:, bi, g0:g1, :],
                            x16[:, bi, g0:g1, 0:_N],
                            x16[:, bi, g0:g1, 2 : 2 + _N],
                        )
                for bi in range(_BPC):
                    for h, (g0, g1) in enumerate(_HALVES):
                        if final:
                            nc.scalar.activation(
                                p16[:, bi, g0:g1, :], phs[bi, h][:],
                                mybir.ActivationFunctionType.Copy, scale=scale,
                            )
                            nc.vector.scalar_tensor_tensor(
                                x16[:, bi, g0:g1, 1 : 1 + _N],
                                t16[:, bi, g0:g1, :], scale,
                                p16[:, bi, g0:g1, :],
                                mybir.AluOpType.mult, mybir.AluOpType.add,
                            )
                        else:
                            nc.scalar.copy(p16[:, bi, g0:g1, :], phs[bi, h][:])
                            nc.vector.tensor_add(
                                x16[:, bi, g0:g1, 1 : 1 + _N],
                                t16[:, bi, g0:g1, :],
                                p16[:, bi, g0:g1, :],
                            )

            for bi in range(_BPC):
                nc.sync.dma_start(out[:, bi], x16[:, bi, :, 1 : 1 + _N])

    nc.finalize()
    return nc


_NC_CACHE: dict = {}


def _get_nc(maxiter: int):
    if maxiter not in _NC_CACHE:
        _NC_CACHE[maxiter] = _build_nc(maxiter)
    return _NC_CACHE[maxiter]


def _mats16():
    one = np.float16(1.0)
    tm = np.zeros((_P, _P), np.float16)
    i = np.arange(_P - 1)
    tm[i, i + 1] = one
    tm[i + 1, i] = one
    cn = np.zeros((_P, _P), np.float16)
    cn[_P - 1, 0] = one
    cs = np.zeros((_P, _P), np.float16)
    cs[0, _P - 1] = one
    im = np.eye(_P, dtype=np.float16)
    return np.ascontiguousarray(np.stack([im, tm, cn, cs], axis=1))


def _verify_stencil(M_rows, M_cols, M_vals, invD):
    """Check the COO matrix is exactly the uniform -1 4-neighbor stencil
    (no wraps) and invD == 0.25 everywhere."""
    r = np.asarray(M_rows).astype(np.int64)
    c = np.asarray(M_cols).astype(np.int64)
    v = np.asarray(M_vals)
    if not np.all(np.asarray(invD) == np.float32(0.25)):
        return False
    if not np.all(v == np.float32(-1.0)):
        return False
    off = c - r
    bands = {o: off == o for o in (1, -1, _N, -_N)}
    if not (bands[1] | bands[-1] | bands[_N] | bands[-_N]).all():
        return False
    if np.any((r[bands[1]] % _N) == _N - 1) or np.any((r[bands[-1]] % _N) == 0):
        return False
    n2 = _N * _N
    rows2 = np.arange(n2)
    for o, m in bands.items():
        cnt = np.zeros(n2, np.int64)
        np.add.at(cnt, r[m], 1)
        if o == 1:
            want = (rows2 % _N) != _N - 1
        elif o == -1:
            want = (rows2 % _N) != 0
        elif o == _N:
            want = rows2 < n2 - _N
        else:
            want = rows2 >= _N
        if not np.array_equal(cnt, want.astype(np.int64)):
            return False
    return True


def _fallback(u, b, M_rows, M_cols, M_vals, invD, maxiter):
    """Host scipy path -- only taken if inputs are not the expected stencil."""
    from scipy.sparse import coo_matrix

    Bn = u.shape[0]
    n2 = _N * _N
    M = coo_matrix(
        (np.asarray(M_vals), (np.asarray(M_rows), np.asarray(M_cols))),
        shape=(n2, n2),
    ).tocsr()
    x = np.asarray(u).reshape(Bn, -1).astype(np.float32)
    bb = np.asarray(b).astype(np.float32)
    iD = np.asarray(invD).astype(np.float32)
    for _ in range(int(maxiter)):
        x = ((bb - (M @ x.T).T) * iD[None, :]).astype(np.float32)
    return x.reshape(u.shape)


TRACE = False
LAST = None  # BassKernelResults of the most recent run
LAST_NC = None  # Bass module of the most recent run (for TimelineSim)


def kernel(u, b, M_rows, M_cols, M_vals, invD, maxiter):
    global LAST, LAST_NC
    from concourse.bass_utils import run_bass_kernel_spmd

    u = np.asarray(u)
    b = np.asarray(b)
    mi = int(maxiter)

    if not _verify_stencil(M_rows, M_cols, M_vals, invD):
        return _fallback(u, b, M_rows, M_cols, M_vals, invD, maxiter)

    nc = _get_nc(mi)
    LAST_NC = nc
    mats = _mats16()

    Bn = u.shape[0]
    assert Bn == _NCORES * _BPC
    # host-side prep: partition-major fp16, b pre-scaled by 4^j per group step
    u16 = np.ascontiguousarray(
        u.reshape(Bn, _PL, _P, _N).transpose(2, 0, 1, 3)
    ).astype(np.float16)  # [P, Bn, PL, N]
    b4 = b.reshape(Bn, _PL, _P, _N).transpose(2, 0, 1, 3).astype(np.float32)
    bq16 = np.stack(
        [
            np.ascontiguousarray(b4 * (4.0**j)).astype(np.float16)
            for j in range(_G)
        ],
        axis=1,
    )  # [P, G, Bn, PL, N]

    in_maps = []
    for k in range(_NCORES):
        sl = slice(_BPC * k, _BPC * (k + 1))
        in_maps.append(
            {
                "u16": np.ascontiguousarray(u16[:, sl]),
                "bq0": np.ascontiguousarray(bq16[:, 0, sl]),
                "bqr": np.ascontiguousarray(bq16[:, 1:, sl]),
                "mats": mats,
            }
        )

    res = run_bass_kernel_spmd(nc, in_maps, list(range(_NCORES)), trace=TRACE)
    LAST = res
    outs = np.concatenate(
        [res.results[k]["out16"] for k in range(_NCORES)], axis=1
    )  # [P, Bn, PL, N]
    full = (
        np.ascontiguousarray(outs.transpose(1, 2, 0, 3))
        .reshape(u.shape)
        .astype(np.float32)
    )
    return full


# revision 6
# speedup vs baseline: 567.1796x; 1.0074x over previous
"""Trainium2 Bass kernel for batched Jacobi iteration (5-point Laplacian).

Reference computation:
    x <- invD * (b - M x)   repeated `maxiter` times,
where M is the off-diagonal part of the 5-point Laplacian on a 512x512
grid in COO form; for the actual inputs M is exactly the 4-neighbor
stencil with value -1 and invD == 0.25, so

    x_new[r, c] = 0.25 * (b[r, c] + x[r-1,c] + x[r+1,c] + x[r,c-1] + x[r,c+1])

(missing neighbors contribute 0).

Strategy (8 NeuronCores, data parallel over batch B=16 -> 2 per core):
  - whole working set lives in SBUF in fp16 for all iterations
  - grid stored as 4 row planes of (128 partitions = rows, 514 cols with
    zero pad columns); E/W neighbor sums are shifted-AP DVE adds (fp16 2x)
  - N/S coupling + the b term accumulate in PSUM via TensorE matmuls
    (tridiagonal / corner / identity stationaries, one matmul per
    plane-bank -- a matmul's PSUM output must stay within one 2KB bank);
    per-batch PSUM tiles rotate through 2 buffers so TensorE streams
    across iterations; one b-half is instead pre-written to PSUM by
    ScalarE (matmuls accumulate on top) to balance PE vs Act load
  - group rescaling y_j = 4^j * x over groups of G=3 iterations makes the
    per-iteration combine a plain fp16 tensor_add (2x DVE mode): the 1/4^G
    scaling is applied once per group, folded into the ScalarE PSUM->SBUF
    copy (scale=4^-G) and one scalar_tensor_tensor
  - host pre-permutes inputs to partition-major fp16 and pre-scales
    b_j = 4^j b, so no device-side dtype conversions are needed
"""

import sys

sys.path.insert(0, "/opt/trn_rl_repo")

import numpy as np

_N = 512  # grid side
_PL = 4  # row planes per grid
_P = 128  # partitions
_W = _N + 2  # padded row width
_NCORES = 8
_BPC = 2  # batches per core
_G = 3  # iterations per rescale group
_HALVES = ((0, 2), (2, 4))
_ACT_PRELOAD = ((0, 1),)  # (batch, half) whose b term is ScalarE-preloaded
_POOL_TADDS = ((0, 0), (1, 0))  # (batch, half) horizontal adds routed to Pool


def _group_js(maxiter):
    out = []
    left = maxiter
    while left > 0:
        g = min(_G, left)
        for j in range(g):
            out.append((j, g))
        left -= g
    return out


def _build_nc(maxiter: int):
    import concourse.bacc as bacc
    import concourse.mybir as mybir
    from concourse.tile import TileContext

    f32 = mybir.dt.float32
    f16 = mybir.dt.float16

    nc = bacc.Bacc("TRN2", target_bir_lowering=False, debug=False, num_devices=_NCORES)

    u_in = nc.declare_dram_parameter("u16", [_P, _BPC, _PL, _N], f16, isOutput=False)
    b0_in = nc.declare_dram_parameter("bq0", [_P, _BPC, _PL, _N], f16, isOutput=False)
    br_in = nc.declare_dram_parameter(
        "bqr", [_P, _G - 1, _BPC, _PL, _N], f16, isOutput=False
    )
    m_in = nc.declare_dram_parameter("mats", [_P, 4, _P], f16, isOutput=False)
    out = nc.declare_dram_parameter("out16", [_P, _BPC, _PL, _N], f16, isOutput=True)

    with TileContext(nc) as tc:
        with (
            tc.tile_pool(name="const", bufs=1) as const,
            tc.tile_pool(name="state", bufs=1) as state,
            tc.tile_pool(name="psum", bufs=2, space="PSUM") as psum,
        ):
            mats = const.tile([_P, 4, _P], f16, name="mats")
            nc.sync.dma_start(mats[:], m_in[:])
            im16 = mats[:, 0, :]
            tm16 = mats[:, 1, :]
            cn16 = mats[:, 2, :]
            cs16 = mats[:, 3, :]

            x16 = state.tile([_P, _BPC, _PL, _W], f16, name="x16")
            nc.gpsimd.memset(x16[:], 0.0)
            bq = state.tile([_P, _G, _BPC, _PL, _N], f16, name="bq")
            t16 = state.tile([_P, _BPC, _PL, _N], f16, name="t16")
            p16 = state.tile([_P, _BPC, _PL, _N], f16, name="p16")

            nc.sync.dma_start(x16[:, :, :, 1 : 1 + _N], u_in[:])
            nc.sync.dma_start(bq[:, 0], b0_in[:])
            for j in range(1, _G):
                nc.sync.dma_start(bq[:, j], br_in[:, j - 1])

            js = _group_js(maxiter)
            for it, (j, glen) in enumerate(js):
                final = j == glen - 1
                scale = 0.25**glen
                phs = {}
                for bi in range(_BPC):
                    for h, (g0, g1) in enumerate(_HALVES):
                        pre = (bi, h) in _ACT_PRELOAD
                        p = psum.tile(
                            [_P, g1 - g0, _N], f32, name=f"p{bi}", tag=f"p{bi}"
                        )
                        phs[bi, h] = p
                        if pre:
                            nc.scalar.copy(p[:], bq[:, j, bi, g0:g1, :])
                        mms = []  # (stationary, rhs, dst plane slot)
                        for g in range(g0, g1):
                            s = g - g0
                            if not pre:
                                mms.append((im16, bq[:, j, bi, g, :], s))
                            mms.append((tm16, x16[:, bi, g, 1 : 1 + _N], s))
                            if g > 0:
                                mms.append(
                                    (cn16, x16[:, bi, g - 1, 1 : 1 + _N], s)
                                )
                            if g < _PL - 1:
                                mms.append(
                                    (cs16, x16[:, bi, g + 1, 1 : 1 + _N], s)
                                )
                        started = set()
                        lasts = {}
                        for i, (_, _, slot) in enumerate(mms):
                            lasts[slot] = i
                        for i, (mat, rhs, slot) in enumerate(mms):
                            nc.tensor.matmul(
                                p[:, slot, :], mat, rhs,
                                start=(slot not in started) and not pre,
                                stop=(lasts[slot] == i),
                                skip_group_check=True,
                            )
                            started.add(slot)
                for bi in range(_BPC):
                    for h, (g0, g1) in enumerate(_HALVES):
                        teng = (
                            nc.gpsimd if (bi, h) in _POOL_TADDS else nc.vector
                        )
                        teng.tensor_add(
                            t16[:, bi, g0:g1, :],
                            x16[:, bi, g0:g1, 0:_N],
                            x16[:, bi, g0:g1, 2 : 2 + _N],
                        )
                for bi in range(_BPC):
                    for h, (g0, g1) in enumerate(_HALVES):
                        if final:
                            nc.scalar.activation(
                                p16[:, bi, g0:g1, :], phs[bi, h][:],
                                mybir.ActivationFunctionType.Copy, scale=scale,
                            )
                            nc.vector.scalar_tensor_tensor(
                                x16[:, bi, g0:g1, 1 : 1 + _N],
                                t16[:, bi, g0:g1, :], scale,
                                p16[:, bi, g0:g1, :],
                                mybir.AluOpType.mult, mybir.AluOpType.add,
                            )
                        else:
                            nc.scalar.copy(p16[:, bi, g0:g1, :], phs[bi, h][:])
                            nc.vector.tensor_add(
                                x16[:, bi, g0:g1, 1 : 1 + _N],
                                t16[:, bi, g0:g1, :],
                                p16[:, bi, g0:g1, :],
                            )

            for bi in range(_BPC):
                nc.sync.dma_start(out[:, bi], x16[:, bi, :, 1 : 1 + _N])

    nc.finalize()
    return nc


_NC_CACHE: dict = {}


def _get_nc(maxiter: int):
    if maxiter not in _NC_CACHE:
        _NC_CACHE[maxiter] = _build_nc(maxiter)
    return _NC_CACHE[maxiter]


def _mats16():
    one = np.float16(1.0)
    tm = np.zeros((_P, _P), np.float16)
    i = np.arange(_P - 1)
    tm[i, i + 1] = one
    tm[i + 1, i] = one
    cn = np.zeros((_P, _P), np.float16)
    cn[_P - 1, 0] = one
    cs = np.zeros((_P, _P), np.float16)
    cs[0, _P - 1] = one
    im = np.eye(_P, dtype=np.float16)
    return np.ascontiguousarray(np.stack([im, tm, cn, cs], axis=1))


def _verify_stencil(M_rows, M_cols, M_vals, invD):
    """Check the COO matrix is exactly the uniform -1 4-neighbor stencil
    (no wraps) and invD == 0.25 everywhere."""
    r = np.asarray(M_rows).astype(np.int64)
    c = np.asarray(M_cols).astype(np.int64)
    v = np.asarray(M_vals)
    if not np.all(np.asarray(invD) == np.float32(0.25)):
        return False
    if not np.all(v == np.float32(-1.0)):
        return False
    off = c - r
    bands = {o: off == o for o in (1, -1, _N, -_N)}
    if not (bands[1] | bands[-1] | bands[_N] | bands[-_N]).all():
        return False
    if np.any((r[bands[1]] % _N) == _N - 1) or np.any((r[bands[-1]] % _N) == 0):
        return False
    n2 = _N * _N
    rows2 = np.arange(n2)
    for o, m in bands.items():
        cnt = np.zeros(n2, np.int64)
        np.add.at(cnt, r[m], 1)
        if o == 1:
            want = (rows2 % _N) != _N - 1
        elif o == -1:
            want = (rows2 % _N) != 0
        elif o == _N:
            want = rows2 < n2 - _N
        else:
            want = rows2 >= _N
        if not np.array_equal(cnt, want.astype(np.int64)):
            return False
    return True


def _fallback(u, b, M_rows, M_cols, M_vals, invD, maxiter):
    """Host scipy path -- only taken if inputs are not the expected stencil."""
    from scipy.sparse import coo_matrix

    Bn = u.shape[0]
    n2 = _N * _N
    M = coo_matrix(
        (np.asarray(M_vals), (np.asarray(M_rows), np.asarray(M_cols))),
        shape=(n2, n2),
    ).tocsr()
    x = np.asarray(u).reshape(Bn, -1).astype(np.float32)
    bb = np.asarray(b).astype(np.float32)
    iD = np.asarray(invD).astype(np.float32)
    for _ in range(int(maxiter)):
        x = ((bb - (M @ x.T).T) * iD[None, :]).astype(np.float32)
    return x.reshape(u.shape)


TRACE = False
LAST = None  # BassKernelResults of the most recent run
LAST_NC = None  # Bass module of the most recent run (for TimelineSim)


def kernel(u, b, M_rows, M_cols, M_vals, invD, maxiter):
    global LAST, LAST_NC
    from concourse.bass_utils import run_bass_kernel_spmd

    u = np.asarray(u)
    b = np.asarray(b)
    mi = int(maxiter)

    if not _verify_stencil(M_rows, M_cols, M_vals, invD):
        return _fallback(u, b, M_rows, M_cols, M_vals, invD, maxiter)

    nc = _get_nc(mi)
    LAST_NC = nc
    mats = _mats16()

    Bn = u.shape[0]
    assert Bn == _NCORES * _BPC
    # host-side prep: partition-major fp16, b pre-scaled by 4^j per group step
    u16 = np.ascontiguousarray(
        u.reshape(Bn, _PL, _P, _N).transpose(2, 0, 1, 3)
    ).astype(np.float16)  # [P, Bn, PL, N]
    b4 = b.reshape(Bn, _PL, _P, _N).transpose(2, 0, 1, 3).astype(np.float32)
    bq16 = np.stack(
        [
            np.ascontiguousarray(b4 * (4.0**j)).astype(np.float16)
            for j in range(_G)
        ],
        axis=1,
    )  # [P, G, Bn, PL, N]

    in_maps = []
    for k in range(_NCORES):
        sl = slice(_BPC * k, _BPC * (k + 1))
        in_maps.append(
            {
                "u16": np.ascontiguousarray(u16[:, sl]),
                "bq0": np.ascontiguousarray(bq16[:, 0, sl]),
                "bqr": np.ascontiguousarray(bq16[:, 1:, sl]),
                "mats": mats,
            }
        )

    res = run_bass_kernel_spmd(nc, in_maps, list(range(_NCORES)), trace=TRACE)
    LAST = res
    outs = np.concatenate(
        [res.results[k]["out16"] for k in range(_NCORES)], axis=1
    )  # [P, Bn, PL, N]
    full = (
        np.ascontiguousarray(outs.transpose(1, 2, 0, 3))
        .reshape(u.shape)
        .astype(np.float32)
    )
    return full
